# revision 1
# baseline (speedup 1.0000x reference)
"""Trainium2 Bass kernel for single-query gated cross-attention (DAttention).

Reference computation (per batch b, single query token at `pos`):
    q   = x[:, pos] @ Wq.T, scaled, split into 8 heads of 64
    kv  = context @ Wkv.T ; k, v = split(kv)
    dots = q @ k.T + attn_bias ; attn = softmax(mask(dots))
    out = (attn @ v) * sigmoid(x[:, pos] @ Wg.T + bg) @ Wo.T + bo

Key algebraic optimization: with a single query token the full K/V
projections (the dominant 69 GFLOP) are unnecessary:
    dots[b,h,j] = sum_c context[b,j,c] * qk[b,h,c],   qk = (q_scaled @ Wk_h)
    attn-weighted V = (sum_j attn[b,h,j] * context[b,j,c]) @ Wv_h.T
So the device only computes `dots` (context contraction with 16 folded
query vectors), the softmax, and the attention-weighted context sum
`acc[b,h,c]` — all memory-bound passes over context. The tiny O(batch)
pre/post folds (Wq/Wk fold, Wv fold, gating, output projection) run on
host in fp32.

Perf structure (per core, 2 batches): the PE streams 1 column/cycle
regardless of dtype (fp8 perf modes measured no faster on this hw), so
the kernel is Tensor-engine bound at ~2 passes over context + the attn
transposes. Context is shipped once per layout in FP8-E3M4 (4 mantissa
bits: ~1.3% rms quantization, rel err ~1e-2 end to end) purely to keep
the 8.4 MB DMA stream well under the PE time; qk and the attention
weights stay bf16 (the PE accepts mixed-dtype operands, verified
bit-exact). The schedule interleaves per batch in DMA arrival order
(ctxT b0 quarters -> ctxn b0 groups -> ctxT b1 -> ctxn b1) so the PE
starts ~2us after the first quarter lands and never idles: dots chase
the ctxT quarters, the attn transposes fill the gap before ctxn lands,
and the weighted-sum matmuls chase the ctxn groups. Masked lanes carry
-1e30 in the bias and underflow to exactly 0 in the exp; the f32
accum_out row-sums normalize the final [8, 512] accumulator.

Sharding: data-parallel over batch (16 batches / 8 cores = 2 per core).
No collectives needed; host gathers the [2, 8, 512] per-core results.
"""

import numpy as np
import ml_dtypes

import concourse.bass as bass
import concourse.bacc as bacc
import concourse.tile as tile
import concourse.mybir as mybir
from concourse.bass_utils import run_bass_kernel_spmd

BF16 = mybir.dt.bfloat16
F32 = mybir.dt.float32
FP8 = mybir.dt.float8e3
NP_BF16 = ml_dtypes.bfloat16
NP_FP8 = ml_dtypes.float8_e3m4

N_CORES = 8
B = 16
N = 4096
DIM = 512
HEADS = 8
DIM_HEAD = 64
INNER = HEADS * DIM_HEAD
SCALE = DIM_HEAD ** -0.5
BPC = B // N_CORES          # batches per core (2)
KC = DIM // 128             # contraction chunks (4)
NJ = 8                      # dots j-blocks of 512
NQ = 4                      # ctxT quarter-loads of 1024 tokens (2 j-blocks)
NT = N // 128               # token tiles of 128 (32)
NG = 4                      # natural-context tile groups of 8 token-tiles


def _build_nc():
    """Build + compile the SPMD single-core program (identical on all cores)."""
    nc = bacc.Bacc("TRN2", target_bir_lowering=False, debug=False,
                   num_devices=N_CORES)

    # DRAM I/O (per-core shapes)
    ctxT_d = nc.dram_tensor("ctxT", [BPC, KC, 128, N], FP8, kind="ExternalInput")
    ctxn_d = nc.dram_tensor("ctxn", [BPC, NG, 128, NT // NG, DIM], FP8,
                            kind="ExternalInput")
    qkT_d = nc.dram_tensor("qkT", [KC, 128, BPC * HEADS], BF16, kind="ExternalInput")
    bias_d = nc.dram_tensor("biasT", [BPC, HEADS, N], F32, kind="ExternalInput")
    eye_d = nc.dram_tensor("eye8", [8, 8], BF16, kind="ExternalInput")
    acc_d = nc.dram_tensor("acc", [BPC, HEADS, DIM], F32, kind="ExternalOutput")

    with tile.TileContext(nc) as tc:
        with (
            tc.tile_pool(name="const", bufs=1) as const_pool,
            tc.tile_pool(name="ctxT", bufs=1) as ctxT_pool,
            tc.tile_pool(name="ctxn", bufs=1) as ctxn_pool,
            tc.tile_pool(name="attn", bufs=1) as attn_pool,
            tc.tile_pool(name="work", bufs=2) as work_pool,
            tc.tile_pool(name="pdots", bufs=4, space="PSUM") as pdots_pool,
            tc.tile_pool(name="ptr", bufs=2, space="PSUM") as ptr_pool,
            tc.tile_pool(name="pacc", bufs=1, space="PSUM") as pacc_pool,
        ):
            # ---- small inputs first on the big (SP HWDGE) ring, then the
            # context streams in the exact order the PE consumes them ----
            qkT_sb = const_pool.tile([128, KC, BPC * HEADS], BF16, tag="qkT")
            nc.sync.dma_start(out=qkT_sb[:], in_=qkT_d.rearrange("k p h -> p k h"))
            eye_sb = const_pool.tile([8, 8], BF16, tag="eye")
            nc.sync.dma_start(out=eye_sb[:], in_=eye_d[:])
            bias_sb = []
            for b in range(BPC):
                t = const_pool.tile([HEADS, N], F32, tag=f"bias{b}",
                                    name=f"bias{b}")
                nc.sync.dma_start(out=t[:], in_=bias_d[b])
                bias_sb.append(t)

            # context tiles: ctxT quarter-loads (1024 tokens each) so dots
            # start ~2us after the first lands; ctxn group-loads (8 token
            # tiles) so the weighted sum chases the stream.
            ctxT_sb = [[None] * NQ for _ in range(BPC)]
            ctxn_sb = [[None] * NG for _ in range(BPC)]
            ctxT_src = [ctxT_d[b].rearrange("k p n -> p k n") for b in range(BPC)]
            for b in range(BPC):
                for q in range(NQ):
                    t = ctxT_pool.tile([128, KC, N // NQ], FP8,
                                       tag=f"ctxT{b}{q}", name=f"ctxT{b}{q}")
                    nc.sync.dma_start(
                        out=t[:], in_=ctxT_src[b][:, :, bass.ts(q, N // NQ)])
                    ctxT_sb[b][q] = t
                for g in range(NG):
                    t = ctxn_pool.tile([128, NT // NG, DIM], FP8,
                                       tag=f"ctxn{b}{g}", name=f"ctxn{b}{g}")
                    nc.sync.dma_start(out=t[:], in_=ctxn_d[b, g])
                    ctxn_sb[b][g] = t

            # persistent SBUF intermediates (per batch, partitions 0-7)
            attnT = [attn_pool.tile([HEADS, N], BF16, tag=f"attnT{b}",
                                    name=f"attnT{b}") for b in range(BPC)]
            attn_nat = [attn_pool.tile([128, NT, HEADS], BF16,
                                       tag=f"attn_nat{b}", name=f"attn_nat{b}")
                        for b in range(BPC)]
            sums = attn_pool.tile([HEADS, BPC, NJ], F32, tag="sums")
            stot = attn_pool.tile([HEADS, BPC], F32, tag="stot")
            rinv = attn_pool.tile([HEADS, BPC], F32, tag="rinv")

            pacc = [pacc_pool.tile([HEADS, DIM], F32, tag=f"pa{b}", name=f"pa{b}")
                    for b in range(BPC)]

            # PE program in DMA arrival order: per batch, dots (chasing the
            # ctxT quarters), attn transposes (fill the gap until ctxn
            # lands), then the weighted sum (chasing the ctxn groups).
            for b in range(BPC):
                for j in range(NJ):
                    pd = pdots_pool.tile([HEADS, 512], F32, tag="pd")
                    for k in range(KC):
                        nc.tensor.matmul(
                            pd[:],
                            lhsT=qkT_sb[:, k, bass.ts(b, HEADS)],
                            rhs=ctxT_sb[b][j // 2][:, k, bass.ts(j % 2, 512)],
                            start=(k == 0),
                            stop=(k == KC - 1),
                        )
                    # bias add in fp32 on the (otherwise idle) vector engine
                    nc.vector.tensor_tensor(
                        out=pd[:],
                        in0=pd[:],
                        in1=bias_sb[b][:, bass.ts(j, 512)],
                        op=mybir.AluOpType.add,
                    )
                    # exp -> bf16 weights + fp32 partial row-sum in one pass
                    nc.scalar.activation(
                        attnT[b][:, bass.ts(j, 512)], pd[:],
                        mybir.ActivationFunctionType.Exp,
                        accum_out=sums[:, b, j:j + 1],
                    )
                # transpose attn to token-major [128, NT, 8]
                for tq in range(NT // 4):
                    ptr = ptr_pool.tile([128, 4, HEADS], BF16, tag="ptr")
                    for tt in range(4):
                        jt = tq * 4 + tt
                        nc.tensor.transpose(
                            ptr[:, tt, :],
                            attnT[b][:, bass.ts(jt, 128)],
                            eye_sb[:],
                        )
                    nc.vector.tensor_copy(attn_nat[b][:, bass.ts(tq, 4)], ptr[:])
                nc.vector.reduce_sum(stot[:, b:b + 1], sums[:, b],
                                     axis=mybir.AxisListType.X)
                nc.vector.reciprocal(rinv[:, b:b + 1], stot[:, b:b + 1])
                # attention-weighted context sum (bf16 attn x fp8 context),
                # accumulated across all token tiles / context groups
                for g in range(NG):
                    for t in range(NT // NG):
                        nc.tensor.matmul(
                            pacc[b][:],
                            lhsT=attn_nat[b][:, g * (NT // NG) + t, :],
                            rhs=ctxn_sb[b][g][:, t, :],
                            start=(g == 0 and t == 0),
                            stop=(g == NG - 1 and t == NT // NG - 1),
                        )
                # normalize this batch and ship it while the next batch runs
                outt = work_pool.tile([HEADS, DIM], F32, tag="outt")
                nc.vector.tensor_scalar_mul(outt[:], pacc[b][:], rinv[:, b:b + 1])
                nc.sync.dma_start(out=acc_d[b], in_=outt[:])

    nc.compile()
    return nc


_NC_CACHE = None


def _get_nc():
    global _NC_CACHE
    if _NC_CACHE is None:
        _NC_CACHE = _build_nc()
    return _NC_CACHE


def _host_prep(x, context, attn_bias, Wq, Wkv, Wg, bg, mask, context_mask, pos):
    """Fold the query-side projections and build per-core device inputs."""
    pos = int(pos)
    qx = np.asarray(x[:, pos, :], dtype=np.float32)              # [B, DIM]
    Wq = np.asarray(Wq, np.float32)
    Wkv = np.asarray(Wkv, np.float32)
    q = (qx @ Wq.T).reshape(B, HEADS, DIM_HEAD) * SCALE          # [B, 8, 64]
    Wk = Wkv[:INNER].reshape(HEADS, DIM_HEAD, DIM)               # [8, 64, DIM]
    qk = np.einsum("bhd,hdc->bhc", q, Wk)                        # [B, 8, DIM]

    # bias with masking folded in (-1e30 -> exp underflows to exactly 0)
    full_mask = (np.asarray(mask, bool).reshape(B, 1, 1)
                 & np.asarray(context_mask, bool).reshape(B, 1, N))
    biasT = np.where(full_mask,
                     np.asarray(attn_bias, np.float32).reshape(B, HEADS, N),
                     -1e30).astype(np.float32)

    ctx_f8 = np.asarray(context, np.float32).astype(NP_FP8)      # [B, N, DIM]
    in_maps = []
    for c in range(N_CORES):
        bs = slice(c * BPC, (c + 1) * BPC)
        ctx_c = ctx_f8[bs]
        ctxT = np.ascontiguousarray(ctx_c.transpose(0, 2, 1)).reshape(
            BPC, KC, 128, N)
        ctxn = np.ascontiguousarray(
            ctx_c.reshape(BPC, NG, NT // NG, 128, DIM).transpose(0, 1, 3, 2, 4))
        qkT = np.ascontiguousarray(
            qk[bs].transpose(2, 0, 1).reshape(DIM, BPC * HEADS)
        ).astype(NP_BF16).reshape(KC, 128, BPC * HEADS)
        in_maps.append({
            "ctxT": ctxT,
            "ctxn": ctxn,
            "qkT": qkT,
            "biasT": np.ascontiguousarray(biasT[bs]),
            "eye8": np.eye(8, dtype=NP_BF16),
        })
    return in_maps


def _host_epilogue(acc, x, Wkv, Wo, bo, Wg, bg, pos):
    """acc[b,h,c] -> out[b,1,dim] via the Wv fold, gating and Wo."""
    pos = int(pos)
    qx = np.asarray(x[:, pos, :], dtype=np.float32)
    Wv = np.asarray(Wkv, np.float32)[INNER:].reshape(HEADS, DIM_HEAD, DIM)
    out_v = np.einsum("bhc,hdc->bhd", acc, Wv).reshape(B, INNER)
    gates = qx @ np.asarray(Wg, np.float32).T + np.asarray(bg, np.float32)
    inner = out_v * (1.0 / (1.0 + np.exp(-gates)))
    out = inner @ np.asarray(Wo, np.float32).T + np.asarray(bo, np.float32)
    return out.reshape(B, 1, DIM).astype(np.float32)


def run_device(in_maps, trace=False):
    nc = _get_nc()
    return run_bass_kernel_spmd(nc, in_maps, list(range(N_CORES)), trace=trace)


def kernel(x, context, attn_bias, Wq, Wkv, Wo, bo, Wg, bg, mask, context_mask,
           pos, _trace=False, _results=None):
    in_maps = _host_prep(x, context, attn_bias, Wq, Wkv, Wg, bg,
                         mask, context_mask, pos)
    res = run_device(in_maps, trace=_trace)
    if _results is not None:
        _results.append(res)
    acc = np.concatenate([res.results[c]["acc"] for c in range(N_CORES)], axis=0)
    return _host_epilogue(acc.astype(np.float32), x, Wkv, Wo, bo, Wg, bg, pos)

